# revision 20
# baseline (speedup 1.0000x reference)
"""APIQMixer Trainium2 kernel — 8-core data-parallel over the b*t axis.

Per core (nbt=2048 rows, 4 free-dim tiles of 512), transposed layout
(features on partitions, bt on the free dim).  All matmul operands bf16;
PSUM + mixing fp32.  Each matmul pass streams its own rhs (row-band
"riders" do NOT merge on HW; only same-rhs column riders are ~free), so
the schedule minimizes pass count and keeps the PE continuously fed to
hold the 2.4 GHz p-state:

  - L1 hypernet: per (stack, mc) group, 4 rider passes write pair tiles
    [128, 2*NF] spanning 2 PSUM banks; ReLU drains are paired
    [128, 1024] ops rotating ACT/DVE.  Stack5 (agent 20) bakes b1 into
    the matmul via a ones-row (K=33) so its drains pair bias-free.
  - Ally L2: per agent/kc, one h1 pass carries the b rider (cols 0:64 ->
    pwe) + per-agent w (cols 64:128 -> pwa half by parity); one h2 pass
    carries pl2a-w + b2 rider.  Per PAIR: abs [128,512] (ACT), qv-mul
    [128,512] (DVE), and ONE fold pass with stacked identities
    [[I64],[I64]] (K=128) accumulating into pwe[0:64].
  - Enemy L2: as baseline (b/we riders; l2e-w/b2 riders).
  - Mixing of tile t is DEFERRED into tile t+1's issue stream: the
    abs/elu/dot chain runs on ACT/DVE while the PE streams t+1's L1
    passes, so the PE never idles at tile boundaries.
  - elu via exp identity: hidden = max(z, min(exp(z)-1, 0)) — Exp reads
    PSUM directly (depth 3).
  - PSUM map (8 banks): ph pair-tiles x2 (4) | pwe | pl2e | bankA
    (pl2a-w / pq) | pwa.
"""

import numpy as np

import concourse.bass as bass
from concourse import bacc
import concourse.mybir as mybir
from concourse.bass import ds, ts
from concourse.bass_utils import run_bass_kernel_spmd
from concourse.tile import TileContext

F32 = mybir.dt.float32
BF16 = mybir.dt.bfloat16
AF = mybir.ActivationFunctionType
OP = mybir.AluOpType

A, NE = 10, 11
FA = FE = 32
E, H = 64, 256
B, T = 128, 128
BT = B * T
SD = A * FA + NE * FE   # 672
NCORES = 8
NBT = BT // NCORES      # 2048 rows per core
NF = 512                # free-dim tile
NT = NBT // NF          # 4 tiles
NAG = A + NE            # 21 agents

W_NAMES = [
    "l1a_w1", "l1a_w2", "l1e_w1", "l1e_w2",
    "l2a_w1", "l2a_w2", "l2e_w1", "l2e_w2",
]


def _mm(nc, out, lhsT, rhs, **kw):
    nc.tensor.matmul(out, lhsT, rhs, **kw)


def build():
    nc = bacc.Bacc()

    qvT_e = nc.declare_dram_parameter("qvT", [A, NBT], BF16, isOutput=False)
    qvB_e = nc.declare_dram_parameter("qvB", [E, A, NBT], BF16, isOutput=False)
    stT_e = nc.declare_dram_parameter("statesT", [SD, NBT], BF16, isOutput=False)
    w1a4_e = nc.declare_dram_parameter("w1a4", [128, 2 * H], BF16, isOutput=False)
    w1e4_e = nc.declare_dram_parameter("w1e4", [128, 2 * H], BF16, isOutput=False)
    w1e5_e = nc.declare_dram_parameter("w1e5", [33, 2 * H], BF16, isOutput=False)
    w2l1a_e = nc.declare_dram_parameter("w2l1a", [128, 256], BF16, isOutput=False)
    w2l1e_e = nc.declare_dram_parameter("w2l1e", [128, 256], BF16, isOutput=False)
    w2l2a_e = nc.declare_dram_parameter("w2l2a", [128, 256], BF16, isOutput=False)
    w2l2e_e = nc.declare_dram_parameter("w2l2e", [128, 256], BF16, isOutput=False)
    bvec_e = nc.declare_dram_parameter("bvec", [128, 14], F32, isOutput=False)
    cmat_e = nc.declare_dram_parameter("cmat", [128, 385], BF16, isOutput=False)
    out_e = nc.declare_dram_parameter("out", [NBT], F32, isOutput=True)

    from contextlib import ExitStack
    with TileContext(nc) as tc, ExitStack() as ctx:
        const = ctx.enter_context(tc.tile_pool(name="const", bufs=1))
        hpool = ctx.enter_context(tc.tile_pool(name="hpool", bufs=20))
        qbp = ctx.enter_context(tc.tile_pool(name="qbp", bufs=2))
        mix = ctx.enter_context(tc.tile_pool(name="mix", bufs=2))
        ph = ctx.enter_context(tc.tile_pool(name="ph", bufs=2, space="PSUM"))
        pacc = ctx.enter_context(tc.tile_pool(name="pacc", bufs=1, space="PSUM"))

        # ---------------- static loads (first-use order) ----------------
        stacks = [None] * 5
        w1a4 = const.tile([128, 2 * H], BF16, name="w1a4")
        nc.sync.dma_start(out=w1a4[:, :], in_=w1a4_e[:, :])
        bvec = const.tile([128, 14], F32, name="bvec")
        nc.sync.dma_start(out=bvec[:, :], in_=bvec_e[:, :])
        # stacks: load tile-0 column slices first so compute starts early
        for s in range(5):
            stacks[s] = const.tile([128, NBT], BF16, name=f"stack{s}")
        st5 = const.tile([33, NBT], BF16, name="stack5")
        nc.sync.dma_start(out=stacks[0][:, 0:NF], in_=stT_e[0:128, 0:NF])
        w2l1a = const.tile([128, 256], BF16, name="w2l1a")
        nc.sync.dma_start(out=w2l1a[:, :], in_=w2l1a_e[:, :])
        w2l2a = const.tile([128, 256], BF16, name="w2l2a")
        nc.sync.dma_start(out=w2l2a[:, :], in_=w2l2a_e[:, :])
        cmat = const.tile([128, 385], BF16, name="cmat")
        nc.sync.dma_start(out=cmat[:, :], in_=cmat_e[:, :])
        for s in (1, 2):
            nc.sync.dma_start(out=stacks[s][:, 0:NF],
                              in_=stT_e[ds(128 * s, 128), 0:NF])
        w1e4 = const.tile([128, 2 * H], BF16, name="w1e4")
        nc.sync.dma_start(out=w1e4[:, :], in_=w1e4_e[:, :])
        w1e5 = const.tile([33, 2 * H], BF16, name="w1e5")
        nc.sync.dma_start(out=w1e5[:, :], in_=w1e5_e[:, :])
        w2l1e = const.tile([128, 256], BF16, name="w2l1e")
        nc.sync.dma_start(out=w2l1e[:, :], in_=w2l1e_e[:, :])
        w2l2e = const.tile([128, 256], BF16, name="w2l2e")
        nc.sync.dma_start(out=w2l2e[:, :], in_=w2l2e_e[:, :])
        for s in (3, 4):
            nc.sync.dma_start(out=stacks[s][:, 0:NF],
                              in_=stT_e[ds(128 * s, 128), 0:NF])
        nc.sync.dma_start(out=st5[0:32, 0:NF], in_=stT_e[ds(640, 32), 0:NF])
        nc.gpsimd.memset(st5[32:33, :], 1.0)
        qvT = const.tile([A, NBT], BF16, name="qvT")
        nc.sync.dma_start(out=qvT[:, :], in_=qvT_e[:, :])
        # remaining columns of the stacks
        for s in range(5):
            nc.sync.dma_start(out=stacks[s][:, NF:NBT],
                              in_=stT_e[ds(128 * s, 128) if s else ds(0, 128),
                                        NF:NBT])
        nc.sync.dma_start(out=st5[0:32, NF:NBT], in_=stT_e[ds(640, 32), NF:NBT])

        ones10 = cmat[0:A, 128:192]     # [10, 64] all ones
        dotsel = cmat[0:E, 256:321]     # [64, 65]: col 64 = ones
        i64lo = cmat[E:128, 321:385]    # [64@64:128, 64] identity

        b1a_sb = bvec[:, 0:4]
        b1e_sb = bvec[:, 4:8]
        wab_sb = bvec[:, 8:9]
        zb_sb = bvec[0:E, 9:10]
        web_sb = bvec[:, 10:11]
        w2ab_sb = bvec[0:E, 11:12]
        w2eb_sb = bvec[0:E, 12:13]
        ob_sb = bvec[:, 13:14]

        relu_ctr = [0]

        def relu_pair(dst, src, bias_ap):
            # alternate ACT / DVE (GpSimd cannot read PSUM)
            i = relu_ctr[0] % 2
            relu_ctr[0] += 1
            if i == 0:
                nc.scalar.activation(dst, src, AF.Relu, bias=bias_ap)
            else:
                nc.vector.tensor_scalar(dst, src, bias_ap, 0.0, OP.add, OP.max)

        # ------------- per-tile state -------------
        # deferred mixing closures from the previous tile
        pending = {"mixA": None, "mixB": None, "mixB_arg": None}

        def emit_tile(t):
            btsl = ds(NF * t, NF)
            # issue the previous tile's mixing chain first: its ACT/DVE
            # ops overlap this tile's L1 passes on the PE
            if pending["mixA"] is not None:
                pending["mixB_arg"] = pending["mixA"]()
                pending["mixA"] = None
            # persistent per-tile accumulators (pacc bufs=1: banks reused
            # each tile; Tile inserts WAR hazards vs the deferred reads)
            pwe = pacc.tile([128, NF], F32, space="PSUM", name=f"pwe{t}",
                            tag="pwe")
            pl2e = pacc.tile([128, NF], F32, space="PSUM", name=f"pl2e{t}",
                             tag="pl2e")
            b56 = pacc.tile([128, 2 * NF], F32, space="PSUM", name=f"b56{t}",
                            tag="b56")
            pl2a_w = b56[0:E, 0:NF]
            pwa_slot = [b56[E:128, 0:NF], b56[E:128, NF:2 * NF]]
            pq_sl = b56[E:128, 0:NF]     # reused after the allies drain

            ctr = {"b": 0, "we": 0, "l2a": 0, "b2": 0, "l2e": 0}
            N_WE = 2 * NE
            N_L2A = 2 * A
            N_L2E = 2 * NE

            hs = {}        # (s, mc, r) -> (pair tile, half)
            qtall = qbp.tile([128, A, NF], BF16, name=f"qvb_{t}", tag="qvb")
            nc.sync.dma_start(out=qtall[E:128, :, :], in_=qvB_e[:, :, btsl])

            def l1_group(s, mc):
                """One (stack, mc) group: 4 rider passes -> 2 pair tiles,
                2 paired relu drains."""
                if s == 5:
                    # agent 20; bias baked via ones-row, K=33.  One pair
                    # of mc chunks (mc, mc+1) per call.
                    pht = ph.tile([128, 2 * NF], F32, space="PSUM",
                                  name=f"ph_{t}_5_{mc}", tag="ph")
                    ht = hpool.tile([128, 2 * NF], BF16,
                                    name=f"h_{t}_5_{mc}", tag="h")
                    for j in range(2):
                        mcr = mc + j
                        _mm(nc, pht[:, ds(NF * j, NF)],
                            w1e5[0:33, ds(128 * mcr, 128)],
                            st5[0:33, btsl],
                            start=True, stop=True, tile_position=(0, 0))
                        hs[(5, mcr, 0)] = (ht, j)
                    relu_pair(ht[:, :], pht[:, :], 0.0)
                    return
                for pr in range(2):           # rider pairs (0,1), (2,3)
                    pht = ph.tile([128, 2 * NF], F32, space="PSUM",
                                  name=f"ph_{t}_{s}_{mc}_{pr}", tag="ph")
                    ht = hpool.tile([128, 2 * NF], BF16,
                                    name=f"h_{t}_{s}_{mc}_{pr}", tag="h")
                    for j in range(2):
                        r = 2 * pr + j
                        ag = 4 * s + r
                        isally = ag < A
                        _mm(nc, pht[:, ds(NF * j, NF)],
                            (w1a4 if isally else w1e4)[ds(32 * r, 32),
                                                       ds(128 * mc, 128)],
                            stacks[s][ds(32 * r, 32), btsl],
                            start=True, stop=True, tile_position=(32 * r, 0))
                        hs[(s, mc, r)] = (ht, j)
                    # bias col mc (same for both halves of the pair)
                    isally = 4 * s + 2 * pr < A
                    relu_pair(ht[:, :], pht[:, :],
                              (b1a_sb if isally else b1e_sb)[:, ds(mc, 1)])

            def hap(s, mc, r):
                ht, j = hs[(s, mc, r)]
                return ht[:, ds(NF * j, NF)]

            def ally_h1(s, r):
                ag = 4 * s + r
                psl = pwa_slot[ag % 2]
                for kc in range(2):
                    h1 = hap(s, kc, r)
                    _mm(nc, pwe[0:E, :], w2l1a[:, ds(128 * kc + E, E)],
                        h1, start=(ctr["b"] == 0), stop=False,
                        tile_position=(0, 0), skip_group_check=True)
                    ctr["b"] += 1
                    _mm(nc, psl, w2l1a[:, ds(128 * kc, E)],
                        h1, start=(kc == 0), stop=(kc == 1),
                        tile_position=(0, E), skip_group_check=True)

            def ally_h2(s, r):
                for kc in range(2):
                    h2 = hap(s, 2 + kc, r)
                    _mm(nc, pl2a_w, w2l2a[:, ds(128 * kc, E)],
                        h2, start=(ctr["l2a"] == 0),
                        stop=(ctr["l2a"] == N_L2A - 1),
                        tile_position=(0, 0), skip_group_check=True)
                    ctr["l2a"] += 1
                    _mm(nc, pl2e[ds(E, E), :], w2l2a[:, ds(128 * kc + E, E)],
                        h2, start=(ctr["b2"] == 0), stop=False,
                        tile_position=(0, E), skip_group_check=True)
                    ctr["b2"] += 1

            def pair_fold(pair):
                # abs of both agents' w slots (+bias) in one [64,1024] op,
                # qv multiply, then 2 identity-fold passes into pwe[0:64]
                abs_t = mix.tile([128, 2 * NF], BF16, name=f"abs_{t}_{pair}",
                                 tag=f"abs{pair}")
                nc.scalar.activation(abs_t[E:128, :], b56[E:128, :], AF.Abs,
                                     bias=wab_sb[E:128, :])
                prod = mix.tile([128, 2 * NF], BF16, name=f"prod_{t}_{pair}",
                                tag=f"prod{pair}")
                nc.vector.tensor_mul(prod[E:128, :], abs_t[E:128, :],
                                     qtall[E:128, ds(2 * pair, 2), :])
                for half in range(2):
                    _mm(nc, pwe[0:E, :], i64lo,
                        prod[ds(E, E), ds(NF * half, NF)],
                        start=False, stop=False,
                        tile_position=(E, 0), skip_group_check=True)

            def enemy_ag(s, r):
                for kc in range(2):
                    h1 = hap(s, kc, r)
                    _mm(nc, pwe[0:E, :], w2l1e[:, ds(128 * kc, E)],
                        h1, start=(ctr["b"] == 0), stop=False,
                        tile_position=(0, 0), skip_group_check=True)
                    ctr["b"] += 1
                    _mm(nc, pwe[ds(E, E), :], w2l1e[:, ds(128 * kc + E, E)],
                        h1, start=(ctr["we"] == 0),
                        stop=(ctr["we"] == N_WE - 1),
                        tile_position=(0, E), skip_group_check=True)
                    ctr["we"] += 1
                for kc in range(2):
                    h2 = hap(s, 2 + kc, r)
                    _mm(nc, pl2e[0:E, :], w2l2e[:, ds(128 * kc, E)],
                        h2, start=(ctr["l2e"] == 0),
                        stop=(ctr["l2e"] == N_L2E - 1),
                        tile_position=(0, 0), skip_group_check=True)
                    ctr["l2e"] += 1
                    _mm(nc, pl2e[ds(E, E), :], w2l2e[:, ds(128 * kc + E, E)],
                        h2, start=False, stop=False,
                        tile_position=(0, E), skip_group_check=True)

            # ---- deferred mixing closures (emitted during tile t+1) ----
            def mixA():
                we_t = mix.tile([128, NF], BF16, name=f"we_{t}", tag="we")
                nc.scalar.activation(we_t[E:128, :], pwe[E:128, :], AF.Abs,
                                     bias=web_sb[E:128, :])
                he_t = mix.tile([128, NF], BF16, name=f"he_{t}", tag="he")
                nc.vector.tensor_mul(he_t[E:128, :], we_t[E:128, :], pq_sl)
                _mm(nc, pwe[0:E, :], i64lo, he_t[ds(E, E), :],
                    start=False, stop=True, tile_position=(E, 0),
                    skip_group_check=True)
                z = mix.tile([E, NF], F32, name=f"z_{t}", tag="z")
                nc.vector.tensor_scalar(z[:, :], pwe[0:E, :], zb_sb, None,
                                        OP.add)
                texp = mix.tile([E, NF], F32, name=f"texp_{t}", tag="texp")
                nc.scalar.activation(texp[:, :], pwe[0:E, :], AF.Exp,
                                     bias=zb_sb)
                tmp = mix.tile([E, NF], F32, name=f"tmp_{t}", tag="tmp")
                nc.vector.tensor_scalar(tmp[:, :], texp[:, :], -1.0, 0.0,
                                        OP.add, OP.min)
                hidden = mix.tile([E, NF], BF16, name=f"hidden_{t}",
                                  tag="hidden")
                nc.vector.tensor_max(hidden[:, :], tmp[:, :], z[:, :])
                w2a_t = mix.tile([E, NF], BF16, name=f"w2a_{t}", tag="w2a")
                nc.scalar.activation(w2a_t[:, :], pl2a_w, AF.Abs,
                                     bias=w2ab_sb)
                w2e_t = mix.tile([E, NF], BF16, name=f"w2e_{t}", tag="w2e")
                nc.scalar.activation(w2e_t[:, :], pl2e[0:E, :], AF.Abs,
                                     bias=w2eb_sb)
                w2s = mix.tile([E, NF], BF16, name=f"w2s_{t}", tag="w2s")
                nc.gpsimd.tensor_add(w2s[:, :], w2a_t[:, :], w2e_t[:, :])
                prodf = mix.tile([E, NF], BF16, name=f"prodf_{t}",
                                 tag="prodf")
                nc.vector.tensor_mul(prodf[:, :], hidden[:, :], w2s[:, :])
                return prodf

            def mixB(prodf):
                _mm(nc, pl2e[0:E + 1, :], dotsel, prodf[:, :], start=False,
                    stop=True, skip_group_check=True)
                o_sb = mix.tile([128, NF], F32, name=f"o_{t}", tag="o")
                nc.scalar.activation(o_sb[E:E + 1, :], pl2e[E:E + 1, :],
                                     AF.Identity, bias=ob_sb[E:E + 1, :])
                nc.sync.dma_start(out=out_e[btsl].unsqueeze(0),
                                  in_=o_sb[E:E + 1, :])

            def run_pending_B():
                if pending["mixB"] is not None:
                    pending["mixB"](pending["mixB_arg"])
                    pending["mixB"] = None

            # ---------------- issue schedule ----------------
            # si=0: L1 s0 (previous tile's mixA already issued above)
            for mc in range(4):
                l1_group(0, mc)
            # si=1: allies of s0 + L1 s1, dot of t-1 early
            ally_h1(0, 0)
            run_pending_B()
            ally_h2(0, 0)
            l1_group(1, 0)
            ally_h1(0, 1)
            ally_h2(0, 1)
            l1_group(1, 1)
            pair_fold(0)
            ally_h1(0, 2)
            ally_h2(0, 2)
            l1_group(1, 2)
            ally_h1(0, 3)
            ally_h2(0, 3)
            l1_group(1, 3)
            pair_fold(1)
            # si=2: allies of s1 + L1 s2
            ally_h1(1, 0)
            ally_h2(1, 0)
            l1_group(2, 0)
            ally_h1(1, 1)
            ally_h2(1, 1)
            l1_group(2, 1)
            pair_fold(2)
            ally_h1(1, 2)
            ally_h2(1, 2)
            l1_group(2, 2)
            ally_h1(1, 3)
            ally_h2(1, 3)
            l1_group(2, 3)
            pair_fold(3)
            # si=3: allies of s2 (ags 8,9) + enemies ags 10,11 + L1 s3
            ally_h1(2, 0)
            ally_h2(2, 0)
            l1_group(3, 0)
            ally_h1(2, 1)
            ally_h2(2, 1)
            l1_group(3, 1)
            pair_fold(4)
            enemy_ag(2, 2)
            l1_group(3, 2)
            enemy_ag(2, 3)
            l1_group(3, 3)
            # si=4: enemies of s3 + L1 s4
            enemy_ag(3, 0)
            l1_group(4, 0)
            enemy_ag(3, 1)
            l1_group(4, 1)
            enemy_ag(3, 2)
            l1_group(4, 2)
            enemy_ag(3, 3)
            l1_group(4, 3)
            # si=5: enemies of s4 + L1 s5 (2 groups)
            enemy_ag(4, 0)
            l1_group(5, 0)
            enemy_ag(4, 1)
            l1_group(5, 2)
            enemy_ag(4, 2)
            enemy_ag(4, 3)
            # si=6: enemy 20 + pq
            enemy_ag(5, 0)
            _mm(nc, pq_sl, ones10, qvT[:, btsl], start=True, stop=True,
                tile_position=(0, E), skip_group_check=True)
            # defer this tile's mixing into the next tile's stream
            pending["mixA"] = mixA
            pending["mixB"] = mixB

        for t in range(NT):
            emit_tile(t)
        # flush the last tile's mixing
        pending["mixB"](pending["mixA"]())

    return nc


_BUILT = None


def _get_nc():
    global _BUILT
    if _BUILT is None:
        _BUILT = build()
        _BUILT.finalize()
    return _BUILT


def _prep_in_maps(inputs):
    qv = np.ascontiguousarray(np.asarray(inputs["qvals"], dtype=np.float32)).reshape(BT, A)
    st = np.ascontiguousarray(np.asarray(inputs["states"], dtype=np.float32)).reshape(BT, SD)
    f32 = np.float32
    g = {n: np.asarray(inputs[n], dtype=f32) for n in W_NAMES}
    bias = {n: np.asarray(inputs[n], dtype=f32) for n in
            ["l1a_b1", "l1a_b2", "l1e_b1", "l1e_b2",
             "l2a_b1", "l2a_b2", "l2e_b1", "l2e_b2"]}
    w1a4 = np.tile(np.concatenate([g["l1a_w1"], g["l2a_w1"]], axis=1), (4, 1))
    w1e_cat = np.concatenate([g["l1e_w1"], g["l2e_w1"]], axis=1)
    w1e4 = np.tile(w1e_cat, (4, 1))
    # stack5 weights with the b1 bias baked in as a ones-row
    w1e5 = np.concatenate(
        [w1e_cat, np.concatenate([bias["l1e_b1"], bias["l2e_b1"]])[None, :]],
        axis=0)
    w2l1a = np.concatenate([g["l1a_w2"][0:128], g["l1a_w2"][128:256]], axis=1)
    # enemy layer-1 W2 with output cols reordered to [b | w]
    l1e_bw = np.concatenate([g["l1e_w2"][:, E:], g["l1e_w2"][:, :E]], axis=1)
    w2l1e = np.concatenate([l1e_bw[0:128], l1e_bw[128:256]], axis=1)
    def pad_l2(w):  # [256, 65] -> [128, 256] with zero-padded bias cols
        p = np.zeros((256, 128), f32)
        p[:, 0:E] = w[:, :E]
        p[:, E] = w[:, E]
        return np.concatenate([p[0:128], p[128:256]], axis=1)
    w2l2a = pad_l2(g["l2a_w2"])
    w2l2e = pad_l2(g["l2e_w2"])
    bvec = np.zeros((128, 14), f32)
    bvec[:, 0:4] = np.concatenate([bias["l1a_b1"], bias["l2a_b1"]]).reshape(4, 128).T
    bvec[:, 4:8] = np.concatenate([bias["l1e_b1"], bias["l2e_b1"]]).reshape(4, 128).T
    bvec[0:E, 8] = bias["l1a_b2"][:E]
    bvec[E:128, 8] = bias["l1a_b2"][:E]
    bvec[0:E, 9] = A * bias["l1a_b2"][E:] + NE * bias["l1e_b2"][E:]
    bvec[E:128, 10] = NE * bias["l1e_b2"][:E]
    bvec[0:E, 11] = A * bias["l2a_b2"][:E]
    bvec[0:E, 12] = NE * bias["l2e_b2"][:E]
    bvec[E, 13] = A * bias["l2a_b2"][E] + NE * bias["l2e_b2"][E]
    cmat = np.zeros((128, 385), f32)
    cmat[0:A, 128:192] = 1.0                      # ones10
    cmat[0:E, 256 + E] = 1.0                      # dotsel col 64
    cmat[E:128, 321:385] = np.eye(E, dtype=f32)   # i64lo
    import ml_dtypes
    bf16 = ml_dtypes.bfloat16
    wmaps = {
        "w1a4": np.ascontiguousarray(w1a4).astype(bf16),
        "w1e4": np.ascontiguousarray(w1e4).astype(bf16),
        "w1e5": np.ascontiguousarray(w1e5).astype(bf16),
        "w2l1a": np.ascontiguousarray(w2l1a).astype(bf16),
        "w2l1e": np.ascontiguousarray(w2l1e).astype(bf16),
        "w2l2a": np.ascontiguousarray(w2l2a).astype(bf16),
        "w2l2e": np.ascontiguousarray(w2l2e).astype(bf16),
        "bvec": bvec, "cmat": cmat.astype(bf16),
    }
    in_maps = []
    for c in range(NCORES):
        sl = slice(c * NBT, (c + 1) * NBT)
        qvc = np.ascontiguousarray(qv[sl].T).astype(bf16)  # [A, NBT]
        # qv broadcast across 64 partitions: [E, A, NBT]
        qvb = np.ascontiguousarray(
            np.broadcast_to(qvc[None, :, :], (E, A, NBT))).astype(bf16)
        m = {
            "qvT": qvc,
            "qvB": qvb,
            "statesT": np.ascontiguousarray(st[sl].T).astype(bf16),
        }
        m.update(wmaps)
        in_maps.append(m)
    return in_maps


def run(inputs, **kw):
    nc = _get_nc()
    in_maps = _prep_in_maps(inputs)
    res = run_bass_kernel_spmd(nc, in_maps, list(range(NCORES)), **kw)
    out = np.concatenate([
        np.asarray(res.results[i]["out"], dtype=np.float32).reshape(NBT)
        for i in range(NCORES)])
    return out.reshape(B, T, 1), res


def kernel(**inputs):
    out, _ = run(inputs)
    return out
